# revision 13
# baseline (speedup 1.0000x reference)
"""Trainium2 Bass kernel for DPQNetwork vq_codebook forward.

reference:
    r = einsum('bcd,ckd->bck', inputs, centroids_k)        # scores
    BatchNorm1d over C (training stats over (B, K)), then
    mse = r_norm.max(-1); codes = r_norm.argmax(-1); plus centroids passthrough.

Since the BN transform per channel is affine with positive scale (gamma=1 in
this problem), argmax over k commutes with it:
    codes = argmax_k(raw r)
    mse   = (max_k raw r - mean_c) * rsqrt(var_c + eps) * gamma_c + beta_c

Kernel strategy (8 cores, data-parallel over batch):
  - per core: B_loc=1024 rows. Per 128-row tile and per channel c:
    fp32 matmul lhsT=X_c^T[64d,128b], rhs=[C_c^T | sum_k c_k][64d,257] -> PSUM
    column 256 gives sum_k r (for BN mean) for free.
  - DVE segmented reduce_max over PSUM -> per-(b,c) max M.
  - ScalarE Square activation with accum_out -> per-(b,row) sum_k r^2 (BN var).
  - custom DVE op (first-match): accum_out = min_k( k if r>=M else BIG )
    -> exact argmax incl. first-index tie behavior.
  - tiny cross-partition sums via ones-matmul, cross-core AllReduce of the
    128-float stats vector, BN affine applied on device, codes cast to int32.
"""

import sys

sys.path.insert(0, "/opt/trn_rl_repo")

import numpy as np
from contextlib import ExitStack

import concourse.bass as bass
import concourse.bacc as bacc
import concourse.tile as tile
from concourse import mybir
from concourse.bass_utils import run_bass_kernel_spmd

# ---------------------------------------------------------------- constants
B, C, K, D = 8192, 64, 256, 64
N_CORES = 8
B_LOC = B // N_CORES
BT = 128                     # batch rows per tile (partition dim)
N_BT = B_LOC // BT           # b-tiles per core
CHUNK = 4                    # channels per PSUM group (4 banks of 512 fp32)
BN_EPS = 1e-5
BIG = 1.0e9
F32 = mybir.dt.float32
BF16 = mybir.dt.bfloat16
I32 = mybir.dt.int32

# ------------------------------------------------- custom DVE op (argmax)
_ARGMAX_OP = None


def _argmax_first_op():
    """Register (once) a custom DVE op:
        body[k]   = select(in0[k] >= s0, k, s1)
        accum_out = min(s1, min_k body[k])
    s0: per-partition max value; s1: BIG. accum_out = first index where
    in0 == max (exact fp32 compare; min picks the first among ties, matching
    jnp.argmax)."""
    global _ARGMAX_OP
    if _ARGMAX_OP is not None:
        return _ARGMAX_OP
    import concourse.dve_ops as dops
    from concourse.dve_spec import Spec, Src0, C0, C1, select, minn, Idx, lower
    from concourse.dve_uop import DveOpSpec

    name = "ARGMAX_FIRST_ANT"
    for op in dops.OPS:
        if op.name == name:
            _ARGMAX_OP = op
            return op

    def _ref(in0, in1, s0, s1, imm2):
        p = in0.shape[0]
        flat = np.asarray(in0, dtype=np.float32).reshape(p, -1)
        s0a = np.broadcast_to(np.asarray(s0, dtype=np.float32).reshape(p, -1), (p, 1))
        s1a = np.broadcast_to(np.asarray(s1, dtype=np.float32).reshape(-1), (1,))
        idx = np.arange(flat.shape[1], dtype=np.float32)[None, :]
        body = np.where(flat >= s0a, idx, s1a[None, :])
        acc = np.minimum(body.min(axis=1, keepdims=True), s1a[None, :])
        return body.reshape(in0.shape).astype(np.float32), acc.astype(np.float32)

    spec = Spec(body=select(Src0 >= C0, Idx, C1), accum=minn, accum_init=C1,
                reference=_ref)
    row = dops._CUSTOM_DVE_ROW_BASE + len(dops.OPS)
    assert row < 0x20
    dops._SUB_OPCODE_FOR_NAME[name] = row
    shas = {}
    for ver in ("v3", "v4"):
        try:
            s = DveOpSpec(name=name, opcode=row, uops=lower(spec, ver=ver),
                          rd1_en=False)
            shas[ver] = s.sha(ver)
        except Exception:
            pass
    op = dops.DveOp(name, spec, subdim=False, uops_sha=shas)
    dops.OPS.append(op)
    dops.CUSTOM_DVE_SPECS[name] = spec
    _ARGMAX_OP = op
    return op


# ---------------------------------------------------------------- builder
def build_module(b_loc=B_LOC, c_dim=C, k_dim=K, d_dim=D, chunk=CHUNK,
                 n_cores=N_CORES):
    """Build the Bass module (same program for all cores, SPMD)."""
    n_bt = b_loc // BT
    kp1 = k_dim + 1
    n_groups = c_dim // chunk
    argmax_op = _argmax_first_op()

    nc = bacc.Bacc("TRN2", target_bir_lowering=False, debug=False,
                   num_devices=n_cores)

    xt_d = nc.dram_tensor("xt", [d_dim, c_dim, b_loc], F32, kind="ExternalInput")
    ct_d = nc.dram_tensor("ct", [d_dim, c_dim, kp1], F32, kind="ExternalInput")
    gam_d = nc.dram_tensor("gam", [1, c_dim], F32, kind="ExternalInput")
    bet_d = nc.dram_tensor("bet", [1, c_dim], F32, kind="ExternalInput")
    mse_d = nc.dram_tensor("mse", [b_loc, c_dim], F32, kind="ExternalOutput")
    codes_d = nc.dram_tensor("codes", [b_loc, c_dim], I32, kind="ExternalOutput")

    with tile.TileContext(nc) as tc, ExitStack() as ctx:
        sb = ctx.enter_context(tc.tile_pool(name="sb", bufs=1))
        xt_pool = ctx.enter_context(tc.tile_pool(name="xt", bufs=2))
        ps = ctx.enter_context(tc.tile_pool(name="ps", bufs=2, space="PSUM"))
        scratch = ctx.enter_context(tc.tile_pool(name="scr", bufs=2))
        outp = ctx.enter_context(tc.tile_pool(name="outp", bufs=2))
        dram = ctx.enter_context(tc.tile_pool(name="dram", bufs=1, space="DRAM"))

        # ---- persistent SBUF tensors
        ct_sb = sb.tile([d_dim, c_dim, kp1], F32)
        nc.gpsimd.dma_start(ct_sb[:], ct_d.ap())
        m_all = sb.tile([BT, n_bt, c_dim], F32)       # per-(b,c) max
        codes_all = sb.tile([BT, n_bt, c_dim], F32)   # per-(b,c) argmax (fp32)
        ssum = sb.tile([BT, n_bt, c_dim], F32)        # per-(row,bt,c) sum_k r
        ssq = sb.tile([BT, n_bt, c_dim], F32)         # per-(row,bt,c) sum_k r^2
        ones_col = sb.tile([BT, 1], F32)
        nc.vector.memset(ones_col[:], 1.0)
        ones_row = sb.tile([1, BT], F32)
        nc.vector.memset(ones_row[:], 1.0)
        gam_sb = sb.tile([1, c_dim], F32)
        nc.gpsimd.dma_start(gam_sb[:], gam_d.ap())
        bet_sb = sb.tile([1, c_dim], F32)
        nc.gpsimd.dma_start(bet_sb[:], bet_d.ap())

        # ---- main loop
        for bt in range(n_bt):
            xt_t = xt_pool.tile([d_dim, c_dim, BT], F32, tag="xt")
            nc.gpsimd.dma_start(xt_t[:], xt_d.ap()[:, :, bt * BT:(bt + 1) * BT])
            for g in range(n_groups):
                c0 = g * chunk
                pt = ps.tile([BT, chunk, 512], F32, tag="ps")
                for i in range(chunk):
                    c = c0 + i
                    nc.tensor.matmul(pt[:, i, 0:kp1], xt_t[:, c, :],
                                     ct_sb[:, c, :], start=True, stop=True)
                # max over k for the whole group (exact fp32)
                nc.vector.tensor_reduce(
                    m_all[:, bt, c0:c0 + chunk], pt[:, :, 0:k_dim],
                    axis=mybir.AxisListType.X, op=mybir.AluOpType.max)
                # sum_k r: copy the augmented column (batched, strided)
                nc.scalar.activation(
                    ssum[:, bt, c0:c0 + chunk], pt[:, :, k_dim],
                    mybir.ActivationFunctionType.Copy)
                for i in range(chunk):
                    c = c0 + i
                    # sum_k r^2 via Square activation with accumulate
                    sq_scr = scratch.tile([BT, k_dim], BF16, tag="sq")
                    nc.scalar.activation(
                        sq_scr[:], pt[:, i, 0:k_dim],
                        mybir.ActivationFunctionType.Square,
                        accum_out=ssq[:, bt, c:c + 1])
                    # argmax: first k with r == max
                    am_scr = scratch.tile([BT, k_dim], BF16, tag="am")
                    nc.vector._custom_dve(
                        argmax_op, out=am_scr[:], in0=pt[:, i, 0:k_dim],
                        s0=m_all[:, bt, c:c + 1], s1=BIG,
                        accum_out=codes_all[:, bt, c:c + 1])

        # ---- BN statistics: reduce over partitions (b rows) via ones-matmul
        # funnel ssum/ssq through one ACT copy each so the consuming matmul
        # has a single-writer dependency (HW wait-slot limit on LDWEIGHTS)
        stats_cat = sb.tile([BT, 2, n_bt * c_dim], F32)
        nc.scalar.activation(stats_cat[:, 0, :],
                             ssum[:].rearrange("p a b -> p (a b)"),
                             mybir.ActivationFunctionType.Copy)
        nc.scalar.activation(stats_cat[:, 1, :],
                             ssq[:].rearrange("p a b -> p (a b)"),
                             mybir.ActivationFunctionType.Copy)
        stat_ps = ps.tile([1, 2, 512], F32, tag="ps")
        nc.tensor.matmul(stat_ps[0:1, 0, 0:n_bt * c_dim], ones_col[:],
                         stats_cat[:, 0, :], start=True, stop=True)
        nc.tensor.matmul(stat_ps[0:1, 1, 0:n_bt * c_dim], ones_col[:],
                         stats_cat[:, 1, :], start=True, stop=True)
        # fold the n_bt sub-sums: [1, (bt, c)] -> [1, c]
        stat_row = sb.tile([1, 2, c_dim], F32)
        for j in range(2):
            nc.vector.tensor_reduce(
                stat_row[:, j, :],
                stat_ps[0:1, j, 0:n_bt * c_dim].rearrange(
                    "p (a b) -> p b a", a=n_bt),
                axis=mybir.AxisListType.X, op=mybir.AluOpType.add)

        # ---- cross-core AllReduce of the 2*c stats
        ar_in = dram.tile([1, 2 * c_dim], F32)
        ar_out = dram.tile([1, 2 * c_dim], F32)
        nc.gpsimd.dma_start(ar_in[:], stat_row[:].rearrange("p a b -> p (a b)"))
        if n_cores > 1:
            nc.gpsimd.collective_compute(
                "AllReduce", mybir.AluOpType.add,
                replica_groups=[list(range(n_cores))],
                ins=[ar_in[:].opt()], outs=[ar_out[:].opt()])
            nc.gpsimd.dma_start(stat_row[:].rearrange("p a b -> p (a b)"),
                              ar_out[:])

        # ---- finalize BN scale/offset on one partition
        inv_n = 1.0 / float(b_loc * n_cores * k_dim)
        mean_r = sb.tile([1, c_dim], F32)
        nc.vector.tensor_scalar_mul(mean_r[:], stat_row[:, 0, :], inv_n)
        e2_r = sb.tile([1, c_dim], F32)
        nc.vector.tensor_scalar_mul(e2_r[:], stat_row[:, 1, :], inv_n)
        var_r = sb.tile([1, c_dim], F32)
        nc.vector.tensor_mul(var_r[:], mean_r[:], mean_r[:])
        nc.vector.tensor_sub(var_r[:], e2_r[:], var_r[:])
        eps_t = sb.tile([1, 1], F32)
        nc.vector.memset(eps_t[:], BN_EPS)
        sd_r = sb.tile([1, c_dim], F32)
        nc.scalar.activation(sd_r[:], var_r[:],
                             mybir.ActivationFunctionType.Sqrt, bias=eps_t[:])
        so_r = sb.tile([1, 2, c_dim], F32)   # [scale | offset]
        nc.vector.reciprocal(so_r[:, 0, :], sd_r[:])
        nc.vector.tensor_mul(so_r[:, 0, :], so_r[:, 0, :], gam_sb[:])
        # offset = beta - mean * scale
        nc.vector.tensor_mul(so_r[:, 1, :], mean_r[:], so_r[:, 0, :])
        nc.vector.tensor_sub(so_r[:, 1, :], bet_sb[:], so_r[:, 1, :])

        # ---- broadcast scale/offset across partitions: ones^T @ row
        bc_ps = ps.tile([BT, 2, 512], F32, tag="ps")
        nc.tensor.matmul(bc_ps[:, 0, 0:2 * c_dim], ones_row[:],
                         so_r[:].rearrange("p a b -> p (a b)"),
                         start=True, stop=True)
        bc_sb = sb.tile([BT, 2, c_dim], F32)
        nc.vector.tensor_copy(bc_sb[:].rearrange("p a b -> p (a b)"),
                              bc_ps[:, 0, 0:2 * c_dim])

        # ---- outputs: mse = M*scale + offset ; codes -> int32
        for bt in range(n_bt):
            mse_t = outp.tile([BT, c_dim], F32, tag="mse")
            nc.vector.tensor_mul(mse_t[:], m_all[:, bt, :], bc_sb[:, 0, :])
            nc.vector.tensor_add(mse_t[:], mse_t[:], bc_sb[:, 1, :])
            nc.gpsimd.dma_start(mse_d.ap()[bt * BT:(bt + 1) * BT, :], mse_t[:])
            codes_t = outp.tile([BT, c_dim], I32, tag="codes")
            nc.vector.tensor_copy(codes_t[:], codes_all[:, bt, :])
            nc.gpsimd.dma_start(codes_d.ap()[bt * BT:(bt + 1) * BT, :], codes_t[:])

    nc.compile()
    return nc


# ---------------------------------------------------------------- host side
def _prep_core_inputs(x_loc, centroids_k, bn_gamma, bn_beta):
    """x_loc: (B_loc, C, D) float32 -> feed dict for one core."""
    c_dim, k_dim, d_dim = centroids_k.shape
    xt = np.ascontiguousarray(x_loc.transpose(2, 1, 0))          # (D, C, B_loc)
    ct = np.empty((d_dim, c_dim, k_dim + 1), dtype=np.float32)   # (D, C, K+1)
    ct[:, :, :k_dim] = centroids_k.transpose(2, 0, 1)
    ct[:, :, k_dim] = centroids_k.sum(axis=1).T                  # sum_k c_k
    return {
        "xt": xt,
        "ct": ct,
        "gam": np.asarray(bn_gamma, dtype=np.float32).reshape(1, c_dim),
        "bet": np.asarray(bn_beta, dtype=np.float32).reshape(1, c_dim),
    }


_NC_CACHE = {}
LAST_RESULT = None


def kernel(inputs, centroids_k, bn_gamma, bn_beta):
    global LAST_RESULT
    inputs = np.asarray(inputs, dtype=np.float32)
    centroids_k = np.asarray(centroids_k, dtype=np.float32)
    b = inputs.shape[0]
    b_loc = b // N_CORES

    key = (b_loc,) + tuple(centroids_k.shape)
    if key not in _NC_CACHE:
        _NC_CACHE[key] = build_module(b_loc=b_loc, c_dim=centroids_k.shape[0],
                                      k_dim=centroids_k.shape[1],
                                      d_dim=centroids_k.shape[2])
    nc = _NC_CACHE[key]

    in_maps = [
        _prep_core_inputs(inputs[i * b_loc:(i + 1) * b_loc], centroids_k,
                          bn_gamma, bn_beta)
        for i in range(N_CORES)
    ]
    res = run_bass_kernel_spmd(nc, in_maps, list(range(N_CORES)))
    LAST_RESULT = res
    codes = np.concatenate([res.results[i]["codes"] for i in range(N_CORES)], axis=0)
    mse = np.concatenate([res.results[i]["mse"] for i in range(N_CORES)], axis=0)
    return codes.astype(np.int32), mse.astype(np.float32), centroids_k
